# revision 21
# baseline (speedup 1.0000x reference)
"""Multi-Head Latent Attention (MLA) Trainium2 kernel — v5.

Problem (hardcoded): B=2, S=2048, D_MODEL=2048, H=16, HEAD_DIM=128,
D_LATENT=512 (D_QK=256 / D_V=256), ROPE_DIM=64, fp32 in/out.

Reference semantics: q = concat([q_no_rope(1024), q_rope(1024)]).reshape(16
heads x 128), so heads 0-7 take both 64-dim halves from the latent
decompression and heads 8-15 take both halves from the rope projection of x;
RoPE rotates dims 64:128 of every head.

Sharding: 8 cores = 2 batches x 4 head-groups; core (b, hg) owns heads
[2hg, 2hg+1, 8+2hg, 8+2hg+1] (2 decompression + 2 rope-projection heads),
computes the shared latent for its batch redundantly, and produces a partial
output projection (its heads' rows of W_out), transposed [e, q]. The host
sums the 4 partials per batch (in f32; device emits bf16 partials).

v5 design notes (measured on HW: v2=351.5us, v3=360.8, v4=370.8):
  - PE matmuls stream at 215ns issue-to-issue; ~292us of matmul issue is
    the floor. ACT exp on [128,512] measured ~716ns (290 fixed + 426
    compute at 1.2GHz), so a bare attention group is ACT-bound: 16 exps =
    11.4us vs 7.1us of PE. v2-v4 all paid distributed PE stalls for this.
  - exp PAIRING: scores for kc pairs write the two 512-wide halves of one
    [128,1024] psum tile (two adjacent banks) and ONE exp activation
    covers both: 8x1143ns = 9.1us ACT per group. With >=2us of woven PE
    filler per group the whole attention phase is PE-bound.
  - v decompression also runs in [128,1024] pairs (v_nat2[j] holds kc=2j
    and 2j+1 side by side; av slices columns), 6 pairs serial + 2 as weave
    units inside group 0.
  - single weave queue of PE filler units (v pairs, the 16 q/k
    decompression pairs in groups 0-3 meeting the rope deadlines, then
    each q-chunk's out-proj m-tiles), popped at EVENLY SPACED slots across
    each group's pair stream (v4 front-loaded the budget and left group
    tails ACT-bound).
  - rope processes head PAIRS with full-width [128,S] DVE ops + DMA
    write-back (3.6us DVE per pair vs 7.2 for [64,S] ops).
  - softmax denominator: eager DVE add-tree (lvl1 halves of each exp pair,
    lvl3 right after the last lvl2), bf16 ones-colsum matmul at group end.
  - barrier-free phase transition: stage-1 pools freed immediately (their
    release only waits stage-1 ops; the last psum group's copies split
    ACT/DVE to halve the drain), attention pools allocated, v pairs run
    inside them.
  - all attention-phase psum->SBUF copies on DVE; ACT does exps only; wo
    loads on the Sync queue so they never delay the first exps.
"""

import math

import numpy as np

B = 2
S = 2048
D = 2048
H4 = 4            # heads per core
HD = 128          # head dim
DL = 512          # d_latent
DQK = 256
RD = 64           # rope dim
NC = 8            # cores

SCALE = 1.0 / math.sqrt(HD)

_prog_cache = {}


def _build_program(phases=4):
    import concourse.tile as tile
    from concourse import bacc, mybir

    bf16 = mybir.dt.bfloat16
    f32 = mybir.dt.float32

    nc = bacc.Bacc("TRN2", target_bir_lowering=False, debug=False, num_devices=1)

    xT = nc.dram_tensor("xT", [D, S], bf16, kind="ExternalInput")
    w_big = nc.dram_tensor("w_big", [D, 1024], bf16, kind="ExternalInput")
    w_qk = nc.dram_tensor("w_qk", [DQK, 512], bf16, kind="ExternalInput")
    w_v = nc.dram_tensor("w_v", [DQK, 512], bf16, kind="ExternalInput")
    w_o = nc.dram_tensor("w_o", [DL, D], bf16, kind="ExternalInput")
    w4_d = nc.dram_tensor("w4", [128, 2048], bf16, kind="ExternalInput")
    w5_d = nc.dram_tensor("w5", [128, 2048], bf16, kind="ExternalInput")
    cos4_d = nc.dram_tensor("cos4", [128, S], bf16, kind="ExternalInput")
    sin4w_d = nc.dram_tensor("sin4w", [128, S], bf16, kind="ExternalInput")
    out_d = nc.dram_tensor("out", [D, S], bf16, kind="ExternalOutput")

    NQ = S // 512    # 4 q chunks of 512 (attention)
    NK = S // 128    # 16 k/seq chunks of 128
    NP = NK // 2     # 8 kc pairs per group
    KD = D // 128    # 16 contraction chunks for stage 1
    N4 = S // 1024   # 2 wide n-chunks of 1024 (stage1)

    with tile.TileContext(nc, pool_alloc_mode="queue") as tc:
        import contextlib

        with contextlib.ExitStack() as ctx:
            # persistent pools (live to end of program, LIFO via ExitStack)
            ones_p = ctx.enter_context(tc.tile_pool(name="onesp", bufs=1))
            qk_p = ctx.enter_context(tc.tile_pool(name="qk", bufs=1))
            v_p = ctx.enter_context(tc.tile_pool(name="vp", bufs=1))

            ones_f32 = ones_p.tile([128, 128], f32)
            nc.gpsimd.memset(ones_f32[:], 1.0)
            ones_bf = ones_p.tile([128, 128], bf16)
            nc.vector.tensor_copy(ones_bf[:], ones_f32[:])
            # per-head assembled q/k: rows 0:64 nr dims, 64:128 roped dims
            # qkT[0..3] = q heads 0..3, qkT[4..7] = k heads 0..3
            qkT = [qk_p.tile([128, S], bf16, name=f"qkT{i}", tag=f"qk{i}")
                   for i in range(8)]
            # v pairs: v_nat2[j] cols 0:512 = kc 2j, cols 512:1024 = kc 2j+1
            v_nat2 = [v_p.tile([128, 1024], bf16, name=f"v{j}", tag=f"v{j}")
                      for j in range(NP)]

            def v_slice(kc, h):
                base = (kc % 2) * 512 + h * 128
                return v_nat2[kc // 2][:, base:base + 128]

            consts_cm = tc.tile_pool(name="consts", bufs=1)
            consts = consts_cm.__enter__()
            swp_cm = tc.tile_pool(name="swpA", bufs=2)
            swp_p = swp_cm.__enter__()
            scr_cm = tc.tile_pool(name="scrA", bufs=3)
            scr_p = scr_cm.__enter__()

            # wdec/lat outlive the stage-1 pools (the weave dec units read
            # them mid-attention) — right-side stack.
            wdec_cm = tc.tile_pool(name="wdec", bufs=1, side="right")
            wdec_p = wdec_cm.__enter__()
            lat_cm = tc.tile_pool(name="lat", bufs=1, side="right")
            lat_p = lat_cm.__enter__()
            xt2_cm = tc.tile_pool(name="xt2", bufs=1, side="right")
            xt2_p = xt2_cm.__enter__()
            w45_cm = tc.tile_pool(name="w45", bufs=1, side="right")
            w45_p = w45_cm.__enter__()
            # latn[l][n4]: latent rows l*128:(l+1)*128, cols n4*1024:+1024
            latn = [[lat_p.tile([128, 1024], bf16, name=f"latT{i}_{n}",
                                tag=f"lat{i}_{n}") for n in range(N4)]
                    for i in range(4)]

            # ---------------- stage 1: bigT = w_big^T @ xT -----------------
            ps1_cm = tc.tile_pool(name="ps1", bufs=8, space="PSUM")
            ps1_p = ps1_cm.__enter__()
            wbig_cm = tc.tile_pool(name="wbig", bufs=1)
            wbig_p = wbig_cm.__enter__()
            xt_cm = tc.tile_pool(name="xt", bufs=18)
            xt_p = xt_cm.__enter__()

            wbig_sb = [wbig_p.tile([128, 1024], bf16, name=f"wb{k}",
                                   tag=f"wb{k}") for k in range(KD)]
            # ---------------- rope helper (head pair, full-width DVE) ------
            # roped rows t[64:128] = raw*cos + swap32(raw)*sin for two head
            # tiles at once: swapped and raw halves of BOTH tiles stacked
            # into [128,S] scratch (DMA), three full-width DVE ops, two
            # DMAs write the rows back.
            def rope_pair(i, j, c0=0, c1=S):
                ti, tj = qkT[i], qkT[j]
                w = c1 - c0
                sw = swp_p.tile([128, S], bf16, name=f"sw{i}_{j}", tag="sw")
                nc.sync.dma_start(sw[0:32, 0:w], ti[96:128, c0:c1])
                nc.sync.dma_start(sw[32:64, 0:w], ti[64:96, c0:c1])
                nc.sync.dma_start(sw[64:96, 0:w], tj[96:128, c0:c1])
                nc.sync.dma_start(sw[96:128, 0:w], tj[64:96, c0:c1])
                raw = scr_p.tile([128, S], bf16, name="raw", tag="scr")
                nc.sync.dma_start(raw[0:64, 0:w], ti[64:128, c0:c1])
                nc.sync.dma_start(raw[64:128, 0:w], tj[64:128, c0:c1])
                tsin = scr_p.tile([128, S], bf16, name="tsin", tag="scr")
                nc.vector.tensor_mul(tsin[:, 0:w], sw[:, 0:w],
                                     sin4w[:, c0:c1])
                res = scr_p.tile([128, S], bf16, name="res", tag="scr")
                nc.vector.tensor_mul(res[:, 0:w], raw[:, 0:w],
                                     cos4[:, c0:c1])
                nc.vector.tensor_add(res[:, 0:w], res[:, 0:w],
                                     tsin[:, 0:w])
                nc.sync.dma_start(ti[64:128, c0:c1], res[0:64, 0:w])
                nc.sync.dma_start(tj[64:128, c0:c1], res[64:128, 0:w])


            rope_early = [False]
            for n4 in range(N4):
                if n4 == 1 and phases >= 4:
                    rope_pair(2, 6, 0, 1024)
                    rope_pair(3, 7, 0, 1024)
                    rope_early[0] = True
                xts = []
                for k in range(KD):
                    if n4 == 0:
                        nc.scalar.dma_start(wbig_sb[k][:],
                                            w_big.ap()[k * 128:(k + 1) * 128, :])
                    if n4 == 0 or phases < 4:
                        x_t = xt_p.tile([128, 1024], bf16, name="xt",
                                        tag="xt")
                    else:
                        # n4=1 tiles outlive stage 1: the deferred m=4/5
                        # chains read them mid-attention
                        x_t = xt2_p.tile([128, 1024], bf16, name=f"x2_{k}",
                                         tag=f"x2_{k}")
                    nc.sync.dma_start(
                        x_t[:], xT.ap()[k * 128:(k + 1) * 128,
                                        n4 * 1024:(n4 + 1) * 1024])
                    xts.append(x_t)
                if n4 == 1 and phases >= 4:
                    xt2_tiles = xts
                if n4 == 0:
                    wqk_sb = []
                    for l in range(2):
                        w_t = wdec_p.tile([128, 512], bf16, name=f"wqk{l}",
                                          tag=f"wqk{l}")
                        nc.scalar.dma_start(w_t[:],
                                            w_qk.ap()[l * 128:(l + 1) * 128, :])
                        wqk_sb.append(w_t)
                    wv_sb = []
                    for l in range(2):
                        w_t = wdec_p.tile([128, 512], bf16, name=f"wv{l}",
                                          tag=f"wv{l}")
                        nc.scalar.dma_start(w_t[:],
                                            w_v.ap()[l * 128:(l + 1) * 128, :])
                        wv_sb.append(w_t)
                    cos4 = consts.tile([128, S], bf16)
                    nc.scalar.dma_start(cos4[:], cos4_d.ap()[:])
                    sin4w = consts.tile([128, S], bf16)
                    nc.scalar.dma_start(sin4w[:], sin4w_d.ap()[:])
                    w4_sb = w45_p.tile([128, 2048], bf16, name="w4",
                                       tag="w4")
                    nc.scalar.dma_start(w4_sb[:], w4_d.ap()[:])
                    w5_sb = w45_p.tile([128, 2048], bf16, name="w5",
                                       tag="w5")
                    nc.scalar.dma_start(w5_sb[:], w5_d.ap()[:])
                for sub in range(2):
                    last_grp = (n4 == N4 - 1 and sub == 1)
                    mlist = (list(range(8)) if (n4 == 0 or phases < 4)
                             else [0, 1, 2, 3, 6, 7])
                    psums = [ps1_p.tile([128, 512], f32, name=f"ps1_{m}",
                                        tag="ps1") for m in mlist]
                    for k in range(KD):
                        for mi, m in enumerate(mlist):
                            nc.tensor.matmul(
                                psums[mi][:],
                                wbig_sb[k][:, m * 128:(m + 1) * 128],
                                xts[k][:, sub * 512:(sub + 1) * 512],
                                start=(k == 0),
                                stop=(k == KD - 1),
                            )
                    lsl = slice(sub * 512, (sub + 1) * 512)
                    nsl = slice(n4 * 1024 + sub * 512,
                                n4 * 1024 + (sub + 1) * 512)
                    for mi, m in enumerate(mlist):
                        dst = (latn[m][n4][:, lsl] if m < 4
                               else qkT[[2, 3, 6, 7][m - 4]][:, nsl])
                        if last_grp and m % 2 == 1:
                            # split the final drain ACT/DVE so the psum
                            # banks recycle fast for the v pairs
                            nc.scalar.copy(dst, psums[mi][:])
                        else:
                            nc.vector.tensor_copy(dst, psums[mi][:])

            # exp-table warm-up on the scalar queue behind the DMA issues
            warm = ones_p.tile([128, 1], f32)
            nc.scalar.activation(warm[:], ones_f32[:, 0:1],
                                 mybir.ActivationFunctionType.Exp)

            if phases == 1:
                for i in range(4):
                    for n in range(N4):
                        nc.sync.dma_start(
                            out_d.ap()[i * 128:(i + 1) * 128,
                                       n * 1024:(n + 1) * 1024],
                            latn[i][n][:])
                for i, t in enumerate(qkT):
                    nc.sync.dma_start(
                        out_d.ap()[512 + i * 128:512 + (i + 1) * 128, :], t[:])

            # debug-path rope (serial, in-place)
            def rope_tiles_dbg(idxs):
                for i in idxs:
                    t = qkT[i]
                    sw = swp_p.tile([64, S], bf16, name=f"swd{i}", tag="sw")
                    nc.sync.dma_start(sw[0:32, :], t[96:128, :])
                    nc.sync.dma_start(sw[32:64, :], t[64:96, :])
                    tmp_sin = scr_p.tile([64, S], bf16, name="tsd", tag="scr")
                    nc.vector.tensor_mul(tmp_sin[0:64, :], sw[0:64, :],
                                         sin4w[0:64, :])
                    tmp_cos = scr_p.tile([64, S], bf16, name="tcd", tag="scr")
                    nc.vector.tensor_mul(tmp_cos[0:64, :], t[64:128, :],
                                         cos4[64:128, :])
                    nc.vector.tensor_add(t[64:128, :], tmp_cos[0:64, :],
                                         tmp_sin[0:64, :])

            if phases == 2 or phases == 3:
                for j in range(NP):
                    for sc in (2 * j, 2 * j + 1):
                        ps = ps1_p.tile([128, 512], f32, name="ps2v",
                                        tag="ps1")
                        for l in range(2):
                            nc.tensor.matmul(
                                ps[:],
                                latn[2 + l][sc // 8][:, (sc % 8) * 128:
                                                     (sc % 8 + 1) * 128],
                                wv_sb[l][:],
                                start=(l == 0), stop=(l == 1),
                            )
                        nc.vector.tensor_copy(
                            v_nat2[j][:, (sc % 2) * 512:(sc % 2 + 1) * 512],
                            ps[:])
                for mt in [0, 2, 1, 3]:
                    for n in range(NQ):
                        nsl = slice(n * 512, (n + 1) * 512)
                        ps = ps1_p.tile([128, 512], f32, name="ps2",
                                        tag="ps1")
                        for l in range(2):
                            nc.tensor.matmul(
                                ps[:],
                                wqk_sb[l][:, mt * 128:(mt + 1) * 128],
                                latn[l][n // 2][:, (n % 2) * 512:
                                                (n % 2 + 1) * 512],
                                start=(l == 0), stop=(l == 1),
                            )
                        nc.vector.tensor_copy(qkT[[0, 1, 4, 5][mt]][:, nsl],
                                              ps[:])
                rope_tiles_dbg([2, 6, 3, 7, 0, 4, 1, 5])
                for i, t in enumerate(qkT):
                    nc.sync.dma_start(out_d.ap()[i * 128:(i + 1) * 128, :],
                                      t[:])
                if phases == 3:
                    for j in range(NP):
                        nc.sync.dma_start(
                            out_d.ap()[1024 + j * 128:1024 + (j + 1) * 128,
                                       0:1024],
                            v_nat2[j][:])

            # free stage-1 pools now: their release only waits stage-1 ops
            xt_cm.__exit__(None, None, None)
            wbig_cm.__exit__(None, None, None)
            ps1_cm.__exit__(None, None, None)

            # ---------------- attention + v-dec + output projection --------
            if phases >= 4:
              with tc.tile_pool(name="wo", bufs=1) as wo_p, \
                 tc.tile_pool(name="exp", bufs=9) as exp_p, \
                 tc.tile_pool(name="den1", bufs=5) as den1_p, \
                 tc.tile_pool(name="den2", bufs=3) as den2_p, \
                 tc.tile_pool(name="den3", bufs=2) as den3_p, \
                 tc.tile_pool(name="acc", bufs=1) as acc_p, \
                 tc.tile_pool(name="ctx", bufs=9) as ctx_p, \
                 tc.tile_pool(name="rden", bufs=1) as rden_p, \
                 tc.tile_pool(name="stage", bufs=3) as stage_p, \
                 tc.tile_pool(name="ps_s", bufs=2, space="PSUM") as ps_s_p, \
                 tc.tile_pool(name="ps_c", bufs=2, space="PSUM") as ps_c_p, \
                 tc.tile_pool(name="ps_o", bufs=2, space="PSUM") as ps_o_p:
                # wo loads on the Sync queue: the ACT queue must reach the
                # first exps with no DMA issues in front of them
                wo_sb = []
                for kk in range(4):
                    w_t = wo_p.tile([128, D], bf16, name=f"wo{kk}",
                                    tag=f"wo{kk}")
                    nc.sync.dma_start(w_t[:],
                                      w_o.ap()[kk * 128:(kk + 1) * 128, :])
                    wo_sb.append(w_t)

                # k-head columns 1024:2048 (key positions, needed from
                # group 0's pair 4): rope the (6,7) pair now; the q-head
                # (2,3) second halves defer until their m=4/5 chains run
                rope_pair(6, 7, 1024, S)

                # v pair: kc 2j,2j+1 -> one [128,1024] psum -> one copy
                def emit_v_pair(j, copy_eng):
                    ps = ps_s_p.tile([128, 1024], f32, name="pss", tag="pss")
                    for half, sc in enumerate((2 * j, 2 * j + 1)):
                        hsl = slice(half * 512, (half + 1) * 512)
                        for l in range(2):
                            nc.tensor.matmul(
                                ps[:, hsl],
                                latn[2 + l][sc // 8][:, (sc % 8) * 128:
                                                     (sc % 8 + 1) * 128],
                                wv_sb[l][:],
                                start=(l == 0), stop=(l == 1),
                            )
                    if copy_eng is nc.scalar:
                        copy_eng.copy(v_nat2[j][:], ps[:])
                    else:
                        copy_eng.tensor_copy(v_nat2[j][:], ps[:])

                # serial v pairs 0..5 (kc 0..11): copies alternate
                # ACT/DVE so the ps_s rotation (and with it group 0's
                # first scores) isn't gated on a single serialized engine
                for j in range(6):
                    emit_v_pair(j, nc.scalar if j % 2 == 0 else nc.vector)

                # ---- weave queue: PE filler units ----
                weave_q = []
                tail_mode = [False]

                def weave(n=1):
                    for _ in range(n):
                        if weave_q:
                            weave_q.pop(0)()

                def mk_v_unit(j):
                    def emit():
                        emit_v_pair(j, nc.vector)
                    return emit

                def mk_dec_unit(mt, n):
                    def emit():
                        nsl = slice(n * 512, (n + 1) * 512)
                        ps = ps_o_p.tile([128, 512], f32, name="pso",
                                         tag="pso")
                        for l in range(2):
                            nc.tensor.matmul(
                                ps[:],
                                wqk_sb[l][:, mt * 128:(mt + 1) * 128],
                                latn[l][n // 2][:, (n % 2) * 512:
                                                (n % 2 + 1) * 512],
                                start=(l == 0), stop=(l == 1),
                            )
                        nc.vector.tensor_copy(qkT[[0, 1, 4, 5][mt]][:, nsl],
                                              ps[:])
                    return emit

                def mk_chain_unit(m, sub):
                    # deferred stage-1: qkT[2 or 3] columns 1024:2048,
                    # one 16-deep chain (3.4us of PE filler)
                    def emit():
                        w_sb = w4_sb if m == 4 else w5_sb
                        ps = ps_o_p.tile([128, 512], f32, name="psx",
                                         tag="pso")
                        for k in range(KD):
                            nc.tensor.matmul(
                                ps[:],
                                w_sb[:, k * 128:(k + 1) * 128],
                                xt2_tiles[k][:, sub * 512:(sub + 1) * 512],
                                start=(k == 0), stop=(k == KD - 1),
                            )
                        dst = qkT[2 if m == 4 else 3]
                        nc.vector.tensor_copy(
                            dst[:, 1024 + sub * 512:1024 + (sub + 1) * 512],
                            ps[:])
                    return emit

                def mk_out_unit(qc, ctx_by_head, m):
                    def emit():
                        qsl = slice(qc * 512, (qc + 1) * 512)
                        ps_o = ps_o_p.tile([128, 512], f32, name="pso",
                                           tag="pso")
                        for kk in range(4):
                            nc.tensor.matmul(
                                ps_o[:],
                                wo_sb[kk][:, m * 128:(m + 1) * 128],
                                ctx_by_head[kk][:],
                                start=(kk == 0), stop=(kk == 3),
                            )
                        st = stage_p.tile([128, 512], bf16, name="stg",
                                          tag="stage")
                        if tail_mode[0]:
                            nc.scalar.copy(st[:], ps_o[:])
                        else:
                            nc.vector.tensor_copy(st[:], ps_o[:])
                        nc.sync.dma_start(
                            out_d.ap()[m * 128:(m + 1) * 128, qsl], st[:])
                    return emit

                # v pairs 6,7 (kc 12..15) pop in group 0 long before
                # those avs; dec pairs at cap 6: qkT[0] done in g0,
                # qkT[4] in g1 -> rope [0,4] after g1 (used g4);
                # qkT[1]/qkT[5] done in g2 -> rope [1,5] after g2 (used
                # g5).
                weave_q.append(mk_v_unit(6))
                weave_q.append(mk_v_unit(7))
                for mt in [0, 2, 1, 3]:
                    for n in range(NQ):
                        weave_q.append(mk_dec_unit(mt, n))
                for m in (4, 5):
                    for sub in range(2):
                        weave_q.append(mk_chain_unit(m, sub))

                NSLOT = 10

                def emit_group(qc, h, wcap):
                    # one (q-chunk, head) attention block: 8 kc-pair slots,
                    # avs lag one pair, weave pops spread evenly across the
                    # NSLOT slots (pre-slot, 8 pair slots, post-tree slot).
                    qsl = slice(qc * 512, (qc + 1) * 512)
                    ps_ctx = ps_c_p.tile([128, 512], f32, name="psc",
                                         tag="psc")
                    exps2 = []
                    dlvl1 = []
                    dlvl2 = []

                    def weave_slot(sj):
                        # ceil-spread: first pop lands at slot 0 so the
                        # group never leads with 4 bare scores matmuls
                        pops = (-((-wcap * (sj + 1)) // NSLOT)
                                - -((-wcap * sj) // NSLOT))
                        for _ in range(pops):
                            if weave_q:
                                weave_q.pop(0)()

                    def exp_half(p, half):
                        return exps2[p][:, half * 512:(half + 1) * 512]

                    def emit_scores_pair(p):
                        ps_s = ps_s_p.tile([128, 1024], f32, name="pss",
                                           tag="pss")
                        for half, kc in enumerate((2 * p, 2 * p + 1)):
                            nc.tensor.matmul(
                                ps_s[:, half * 512:(half + 1) * 512],
                                qkT[4 + h][:, kc * 128:(kc + 1) * 128],
                                qkT[h][:, qsl],
                                start=True, stop=True,
                            )
                        expT = exp_p.tile([128, 1024], bf16, name="expT",
                                          tag="exp")
                        nc.scalar.activation(
                            expT[:], ps_s[:],
                            mybir.ActivationFunctionType.Exp, scale=SCALE)
                        exps2.append(expT)
                        # den tree in full-width bf16 ops (DVE fixed cost
                        # ~270-400ns/op dominates narrow adds)
                        if p % 2 == 1:
                            d = den1_p.tile([128, 1024], bf16, name="d1",
                                            tag="d1")
                            nc.vector.tensor_add(d[:], exps2[p - 1][:],
                                                 exps2[p][:])
                            dlvl1.append(d)
                            if p % 4 == 3:
                                d2 = den2_p.tile([128, 1024], bf16,
                                                 name="d2", tag="d2")
                                nc.vector.tensor_add(
                                    d2[:], dlvl1[p // 4 * 2][:],
                                    dlvl1[p // 4 * 2 + 1][:])
                                dlvl2.append(d2)

                    def emit_av(kc):
                        nc.tensor.matmul(
                            ps_ctx[:],
                            v_slice(kc, h),
                            exp_half(kc // 2, kc % 2),
                            start=(kc == 0), stop=(kc == NK - 1),
                        )

                    weave_slot(0)
                    for p in range(NP):
                        emit_scores_pair(p)
                        if p >= 2:
                            emit_av(2 * p - 4)
                            emit_av(2 * p - 3)
                        weave_slot(p + 1)
                    emit_av(NK - 4)
                    emit_av(NK - 3)
                    # eager den tree finale (wide bf16, then fold halves)
                    d3 = den3_p.tile([128, 1024], bf16, name="d3", tag="d3")
                    nc.vector.tensor_add(d3[:], dlvl2[0][:], dlvl2[1][:])
                    acc = acc_p.tile([128, 512], bf16, name="acc", tag="acc")
                    nc.vector.tensor_add(acc[:], d3[:, 0:512],
                                         d3[:, 512:1024])
                    emit_av(NK - 2)
                    emit_av(NK - 1)
                    weave_slot(NSLOT - 1)
                    ps_den = ps_o_p.tile([128, 512], f32, name="psd",
                                         tag="pso")
                    nc.tensor.matmul(ps_den[:], ones_bf[:], acc[:],
                                     start=True, stop=True)
                    rden = rden_p.tile([128, 512], f32, name="rden",
                                       tag="rden")
                    nc.vector.reciprocal_approx_fast(rden[:], ps_den[:])
                    c_t = ctx_p.tile([128, 512], bf16, name="ctxt",
                                     tag="ctx")
                    nc.vector.tensor_mul(c_t[:], ps_ctx[:], rden[:])
                    if phases == 5:
                        r0 = (qc * 4 + h) * 128
                        nc.sync.dma_start(out_d.ap()[r0:r0 + 128, 0:512],
                                          c_t[:])
                    return c_t

                # x-projection heads first; dec-head groups after their
                # woven decompression + rope.
                order = [(0, 2), (0, 3), (1, 2), (0, 0), (0, 1),
                         (1, 3), (1, 0), (1, 1),
                         (2, 2), (2, 3), (2, 0), (2, 1),
                         (3, 2), (3, 3), (3, 0), (3, 1)]
                caps = [6, 6, 6, 2, 2] + [5] * 11
                ctxs = {}
                for gi, (qc, h) in enumerate(order):
                    ctxs.setdefault(qc, {})[h] = emit_group(qc, h, caps[gi])
                    if gi == 1:
                        rope_pair(0, 4)
                    if gi == 2:
                        rope_pair(1, 5)
                    if gi == 4:
                        # q-head columns 1024:2048 (deferred chains done in
                        # g3/g4); first consumer is (2,2) at g8
                        rope_pair(2, 3, 1024, S)
                    if len(ctxs[qc]) == 4:
                        dct = ctxs.pop(qc)
                        for m in range(16):
                            weave_q.append(mk_out_unit(qc, dct, m))
                tail_mode[0] = True
                while weave_q:
                    weave()
            w45_cm.__exit__(None, None, None)
            xt2_cm.__exit__(None, None, None)
            lat_cm.__exit__(None, None, None)
            wdec_cm.__exit__(None, None, None)
            scr_cm.__exit__(None, None, None)
            swp_cm.__exit__(None, None, None)
            consts_cm.__exit__(None, None, None)

    nc.compile()
    return nc


def _get_program():
    if "nc" not in _prog_cache:
        _prog_cache["nc"] = _build_program()
    return _prog_cache["nc"]


def _host_shards(x, W_comp, W_q_dec, W_k_dec, W_v_dec, W_rope_q, W_rope_k,
                 W_out):
    import ml_dtypes
    bf16 = ml_dtypes.bfloat16

    inv = 1.0 / (10000.0 ** (np.arange(0, RD, 2, dtype=np.float32) / RD))
    ang = np.arange(S, dtype=np.float32)[:, None] * inv[None, :]     # [S, 32]
    cosT = np.cos(ang).T.astype(np.float32)                          # [32, S]
    sinT = np.sin(ang).T.astype(np.float32)
    cos4 = np.ascontiguousarray(np.tile(cosT, (4, 1))).astype(bf16)  # [128,S]
    sin4w = np.ascontiguousarray(np.tile(
        np.concatenate([-sinT, sinT], axis=0), (2, 1))).astype(bf16)  # [128,S]

    in_maps = []
    for c in range(NC):
        b, hg = divmod(c, 4)
        xTb = np.ascontiguousarray(x[b].T.astype(bf16))
        w_big = np.ascontiguousarray(np.concatenate(
            [W_comp,
             W_rope_q[:, hg * 256:(hg + 1) * 256],
             W_rope_k[:, hg * 256:(hg + 1) * 256]], axis=1).astype(bf16))
        w_qk = np.ascontiguousarray(np.concatenate(
            [W_q_dec[:, hg * 256:(hg + 1) * 256],
             W_k_dec[:, hg * 256:(hg + 1) * 256]], axis=1).astype(bf16))
        w_v = np.ascontiguousarray(np.concatenate(
            [W_v_dec[:, hg * 256:(hg + 1) * 256],
             W_v_dec[:, 1024 + hg * 256:1024 + (hg + 1) * 256]],
            axis=1).astype(bf16))
        w_o = np.ascontiguousarray(np.concatenate(
            [W_out[hg * 256:(hg + 1) * 256, :],
             W_out[1024 + hg * 256:1024 + (hg + 1) * 256, :]],
            axis=0).astype(bf16))
        w4 = np.ascontiguousarray(np.concatenate(
            [w_big[k * 128:(k + 1) * 128, 512:640] for k in range(16)],
            axis=1))
        w5 = np.ascontiguousarray(np.concatenate(
            [w_big[k * 128:(k + 1) * 128, 640:768] for k in range(16)],
            axis=1))
        in_maps.append({
            "xT": xTb, "w_big": w_big, "w_qk": w_qk, "w_v": w_v, "w_o": w_o,
            "w4": w4, "w5": w5, "cos4": cos4, "sin4w": sin4w,
        })
    return in_maps


def kernel(x, W_comp, W_q_dec, W_k_dec, W_v_dec, W_rope_q, W_rope_k, W_out,
           _trace=False):
    from concourse import bass_utils

    x = np.asarray(x, np.float32)
    args = [np.asarray(a, np.float32)
            for a in (W_comp, W_q_dec, W_k_dec, W_v_dec,
                      W_rope_q, W_rope_k, W_out)]
    in_maps = _host_shards(x, *args)
    nc = _get_program()
    res = bass_utils.run_bass_kernel_spmd(
        nc, in_maps, core_ids=list(range(NC)), trace=_trace)
    out = np.zeros((B, S, D), np.float32)
    for c in range(NC):
        b = c // 4
        out[b] += res.results[c]["out"].astype(np.float32).T
    if _trace:
        kernel.last_exec_ns = res.exec_time_ns
    return out


# revision 22
# speedup vs baseline: 1.0095x; 1.0095x over previous
"""Multi-Head Latent Attention (MLA) Trainium2 kernel — v5.

Problem (hardcoded): B=2, S=2048, D_MODEL=2048, H=16, HEAD_DIM=128,
D_LATENT=512 (D_QK=256 / D_V=256), ROPE_DIM=64, fp32 in/out.

Reference semantics: q = concat([q_no_rope(1024), q_rope(1024)]).reshape(16
heads x 128), so heads 0-7 take both 64-dim halves from the latent
decompression and heads 8-15 take both halves from the rope projection of x;
RoPE rotates dims 64:128 of every head.

Sharding: 8 cores = 2 batches x 4 head-groups; core (b, hg) owns heads
[2hg, 2hg+1, 8+2hg, 8+2hg+1] (2 decompression + 2 rope-projection heads),
computes the shared latent for its batch redundantly, and produces a partial
output projection (its heads' rows of W_out), transposed [e, q]. The host
sums the 4 partials per batch (in f32; device emits bf16 partials).

v5 design notes (measured on HW: v2=351.5us, v3=360.8, v4=370.8):
  - PE matmuls stream at 215ns issue-to-issue; ~292us of matmul issue is
    the floor. ACT exp on [128,512] measured ~716ns (290 fixed + 426
    compute at 1.2GHz), so a bare attention group is ACT-bound: 16 exps =
    11.4us vs 7.1us of PE. v2-v4 all paid distributed PE stalls for this.
  - exp PAIRING: scores for kc pairs write the two 512-wide halves of one
    [128,1024] psum tile (two adjacent banks) and ONE exp activation
    covers both: 8x1143ns = 9.1us ACT per group. With >=2us of woven PE
    filler per group the whole attention phase is PE-bound.
  - v decompression also runs in [128,1024] pairs (v_nat2[j] holds kc=2j
    and 2j+1 side by side; av slices columns), 6 pairs serial + 2 as weave
    units inside group 0.
  - single weave queue of PE filler units (v pairs, the 16 q/k
    decompression pairs in groups 0-3 meeting the rope deadlines, then
    each q-chunk's out-proj m-tiles), popped at EVENLY SPACED slots across
    each group's pair stream (v4 front-loaded the budget and left group
    tails ACT-bound).
  - rope processes head PAIRS with full-width [128,S] DVE ops + DMA
    write-back (3.6us DVE per pair vs 7.2 for [64,S] ops).
  - softmax denominator: eager DVE add-tree (lvl1 halves of each exp pair,
    lvl3 right after the last lvl2), bf16 ones-colsum matmul at group end.
  - barrier-free phase transition: stage-1 pools freed immediately (their
    release only waits stage-1 ops; the last psum group's copies split
    ACT/DVE to halve the drain), attention pools allocated, v pairs run
    inside them.
  - all attention-phase psum->SBUF copies on DVE; ACT does exps only; wo
    loads on the Sync queue so they never delay the first exps.
"""

import math

import numpy as np

B = 2
S = 2048
D = 2048
H4 = 4            # heads per core
HD = 128          # head dim
DL = 512          # d_latent
DQK = 256
RD = 64           # rope dim
NC = 8            # cores

SCALE = 1.0 / math.sqrt(HD)

_prog_cache = {}


def _build_program(phases=4):
    import concourse.tile as tile
    from concourse import bacc, mybir

    bf16 = mybir.dt.bfloat16
    f32 = mybir.dt.float32

    nc = bacc.Bacc("TRN2", target_bir_lowering=False, debug=False, num_devices=1)

    xT = nc.dram_tensor("xT", [D, S], bf16, kind="ExternalInput")
    w_big = nc.dram_tensor("w_big", [D, 1024], bf16, kind="ExternalInput")
    w_qk = nc.dram_tensor("w_qk", [DQK, 512], bf16, kind="ExternalInput")
    w_v = nc.dram_tensor("w_v", [DQK, 512], bf16, kind="ExternalInput")
    w_o = nc.dram_tensor("w_o", [DL, D], bf16, kind="ExternalInput")
    w4_d = nc.dram_tensor("w4", [128, 2048], bf16, kind="ExternalInput")
    w5_d = nc.dram_tensor("w5", [128, 2048], bf16, kind="ExternalInput")
    cos4_d = nc.dram_tensor("cos4", [128, S], bf16, kind="ExternalInput")
    sin4w_d = nc.dram_tensor("sin4w", [128, S], bf16, kind="ExternalInput")
    out_d = nc.dram_tensor("out", [D, S], bf16, kind="ExternalOutput")

    NQ = S // 512    # 4 q chunks of 512 (attention)
    NK = S // 128    # 16 k/seq chunks of 128
    NP = NK // 2     # 8 kc pairs per group
    KD = D // 128    # 16 contraction chunks for stage 1
    N4 = S // 1024   # 2 wide n-chunks of 1024 (stage1)

    with tile.TileContext(nc, pool_alloc_mode="queue") as tc:
        import contextlib

        with contextlib.ExitStack() as ctx:
            # persistent pools (live to end of program, LIFO via ExitStack)
            ones_p = ctx.enter_context(tc.tile_pool(name="onesp", bufs=1))
            qk_p = ctx.enter_context(tc.tile_pool(name="qk", bufs=1))
            v_p = ctx.enter_context(tc.tile_pool(name="vp", bufs=1))

            ones_f32 = ones_p.tile([128, 128], f32)
            nc.gpsimd.memset(ones_f32[:], 1.0)
            ones_bf = ones_p.tile([128, 128], bf16)
            nc.vector.tensor_copy(ones_bf[:], ones_f32[:])
            # per-head assembled q/k: rows 0:64 nr dims, 64:128 roped dims
            # qkT[0..3] = q heads 0..3, qkT[4..7] = k heads 0..3
            qkT = [qk_p.tile([128, S], bf16, name=f"qkT{i}", tag=f"qk{i}")
                   for i in range(8)]
            # v pairs: v_nat2[j] cols 0:512 = kc 2j, cols 512:1024 = kc 2j+1
            v_nat2 = [v_p.tile([128, 1024], bf16, name=f"v{j}", tag=f"v{j}")
                      for j in range(NP)]

            def v_slice(kc, h):
                base = (kc % 2) * 512 + h * 128
                return v_nat2[kc // 2][:, base:base + 128]

            consts_cm = tc.tile_pool(name="consts", bufs=1)
            consts = consts_cm.__enter__()
            swp_cm = tc.tile_pool(name="swpA", bufs=2)
            swp_p = swp_cm.__enter__()
            scr_cm = tc.tile_pool(name="scrA", bufs=3)
            scr_p = scr_cm.__enter__()

            # wdec/lat outlive the stage-1 pools (the weave dec units read
            # them mid-attention) — right-side stack.
            wdec_cm = tc.tile_pool(name="wdec", bufs=1, side="right")
            wdec_p = wdec_cm.__enter__()
            lat_cm = tc.tile_pool(name="lat", bufs=1, side="right")
            lat_p = lat_cm.__enter__()
            xt2_cm = tc.tile_pool(name="xt2", bufs=1, side="right")
            xt2_p = xt2_cm.__enter__()
            w45_cm = tc.tile_pool(name="w45", bufs=1, side="right")
            w45_p = w45_cm.__enter__()
            # latn[l][n4]: latent rows l*128:(l+1)*128, cols n4*1024:+1024
            latn = [[lat_p.tile([128, 1024], bf16, name=f"latT{i}_{n}",
                                tag=f"lat{i}_{n}") for n in range(N4)]
                    for i in range(4)]

            # ---------------- stage 1: bigT = w_big^T @ xT -----------------
            ps1_cm = tc.tile_pool(name="ps1", bufs=8, space="PSUM")
            ps1_p = ps1_cm.__enter__()
            wbig_cm = tc.tile_pool(name="wbig", bufs=1)
            wbig_p = wbig_cm.__enter__()
            xt_cm = tc.tile_pool(name="xt", bufs=18)
            xt_p = xt_cm.__enter__()

            wbig_sb = [wbig_p.tile([128, 1024], bf16, name=f"wb{k}",
                                   tag=f"wb{k}") for k in range(KD)]
            # ---------------- rope helper (head pair, full-width DVE) ------
            # roped rows t[64:128] = raw*cos + swap32(raw)*sin for two head
            # tiles at once: swapped and raw halves of BOTH tiles stacked
            # into [128,S] scratch (DMA), three full-width DVE ops, two
            # DMAs write the rows back.
            def rope_pair(i, j, c0=0, c1=S):
                ti, tj = qkT[i], qkT[j]
                w = c1 - c0
                sw = swp_p.tile([128, S], bf16, name=f"sw{i}_{j}", tag="sw")
                nc.sync.dma_start(sw[0:32, 0:w], ti[96:128, c0:c1])
                nc.sync.dma_start(sw[32:64, 0:w], ti[64:96, c0:c1])
                nc.sync.dma_start(sw[64:96, 0:w], tj[96:128, c0:c1])
                nc.sync.dma_start(sw[96:128, 0:w], tj[64:96, c0:c1])
                raw = scr_p.tile([128, S], bf16, name="raw", tag="scr")
                nc.sync.dma_start(raw[0:64, 0:w], ti[64:128, c0:c1])
                nc.sync.dma_start(raw[64:128, 0:w], tj[64:128, c0:c1])
                tsin = scr_p.tile([128, S], bf16, name="tsin", tag="scr")
                nc.vector.tensor_mul(tsin[:, 0:w], sw[:, 0:w],
                                     sin4w[:, c0:c1])
                res = scr_p.tile([128, S], bf16, name="res", tag="scr")
                nc.vector.tensor_mul(res[:, 0:w], raw[:, 0:w],
                                     cos4[:, c0:c1])
                nc.vector.tensor_add(res[:, 0:w], res[:, 0:w],
                                     tsin[:, 0:w])
                nc.sync.dma_start(ti[64:128, c0:c1], res[0:64, 0:w])
                nc.sync.dma_start(tj[64:128, c0:c1], res[64:128, 0:w])


            rope_early = [False]
            for n4 in range(N4):
                if n4 == 1 and phases >= 4:
                    rope_pair(2, 6, 0, 1024)
                    rope_pair(3, 7, 0, 1024)
                    rope_early[0] = True
                xts = []
                for k in range(KD):
                    if n4 == 0:
                        nc.scalar.dma_start(wbig_sb[k][:],
                                            w_big.ap()[k * 128:(k + 1) * 128, :])
                    if n4 == 0 or phases < 4:
                        x_t = xt_p.tile([128, 1024], bf16, name="xt",
                                        tag="xt")
                    else:
                        # n4=1 tiles outlive stage 1: the deferred m=4/5
                        # chains read them mid-attention
                        x_t = xt2_p.tile([128, 1024], bf16, name=f"x2_{k}",
                                         tag=f"x2_{k}")
                    nc.sync.dma_start(
                        x_t[:], xT.ap()[k * 128:(k + 1) * 128,
                                        n4 * 1024:(n4 + 1) * 1024])
                    xts.append(x_t)
                if n4 == 1 and phases >= 4:
                    xt2_tiles = xts
                if n4 == 0:
                    wqk_sb = []
                    for l in range(2):
                        w_t = wdec_p.tile([128, 512], bf16, name=f"wqk{l}",
                                          tag=f"wqk{l}")
                        nc.scalar.dma_start(w_t[:],
                                            w_qk.ap()[l * 128:(l + 1) * 128, :])
                        wqk_sb.append(w_t)
                    wv_sb = []
                    for l in range(2):
                        w_t = wdec_p.tile([128, 512], bf16, name=f"wv{l}",
                                          tag=f"wv{l}")
                        nc.scalar.dma_start(w_t[:],
                                            w_v.ap()[l * 128:(l + 1) * 128, :])
                        wv_sb.append(w_t)
                    cos4 = consts.tile([128, S], bf16)
                    nc.scalar.dma_start(cos4[:], cos4_d.ap()[:])
                    sin4w = consts.tile([128, S], bf16)
                    nc.scalar.dma_start(sin4w[:], sin4w_d.ap()[:])
                    w4_sb = w45_p.tile([128, 2048], bf16, name="w4",
                                       tag="w4")
                    nc.scalar.dma_start(w4_sb[:], w4_d.ap()[:])
                    w5_sb = w45_p.tile([128, 2048], bf16, name="w5",
                                       tag="w5")
                    nc.scalar.dma_start(w5_sb[:], w5_d.ap()[:])
                for sub in range(2):
                    last_grp = (n4 == N4 - 1 and sub == 1)
                    mlist = (list(range(8)) if (n4 == 0 or phases < 4)
                             else [0, 1, 2, 3, 6, 7])
                    psums = [ps1_p.tile([128, 512], f32, name=f"ps1_{m}",
                                        tag="ps1") for m in mlist]
                    for k in range(KD):
                        for mi, m in enumerate(mlist):
                            nc.tensor.matmul(
                                psums[mi][:],
                                wbig_sb[k][:, m * 128:(m + 1) * 128],
                                xts[k][:, sub * 512:(sub + 1) * 512],
                                start=(k == 0),
                                stop=(k == KD - 1),
                            )
                    lsl = slice(sub * 512, (sub + 1) * 512)
                    nsl = slice(n4 * 1024 + sub * 512,
                                n4 * 1024 + (sub + 1) * 512)
                    for mi, m in enumerate(mlist):
                        dst = (latn[m][n4][:, lsl] if m < 4
                               else qkT[[2, 3, 6, 7][m - 4]][:, nsl])
                        if last_grp and m % 2 == 1:
                            # split the final drain ACT/DVE so the psum
                            # banks recycle fast for the v pairs
                            nc.scalar.copy(dst, psums[mi][:])
                        else:
                            nc.vector.tensor_copy(dst, psums[mi][:])

            # exp-table warm-up on the scalar queue behind the DMA issues
            warm = ones_p.tile([128, 1], f32)
            nc.scalar.activation(warm[:], ones_f32[:, 0:1],
                                 mybir.ActivationFunctionType.Exp)

            if phases == 1:
                for i in range(4):
                    for n in range(N4):
                        nc.sync.dma_start(
                            out_d.ap()[i * 128:(i + 1) * 128,
                                       n * 1024:(n + 1) * 1024],
                            latn[i][n][:])
                for i, t in enumerate(qkT):
                    nc.sync.dma_start(
                        out_d.ap()[512 + i * 128:512 + (i + 1) * 128, :], t[:])

            # debug-path rope (serial, in-place)
            def rope_tiles_dbg(idxs):
                for i in idxs:
                    t = qkT[i]
                    sw = swp_p.tile([64, S], bf16, name=f"swd{i}", tag="sw")
                    nc.sync.dma_start(sw[0:32, :], t[96:128, :])
                    nc.sync.dma_start(sw[32:64, :], t[64:96, :])
                    tmp_sin = scr_p.tile([64, S], bf16, name="tsd", tag="scr")
                    nc.vector.tensor_mul(tmp_sin[0:64, :], sw[0:64, :],
                                         sin4w[0:64, :])
                    tmp_cos = scr_p.tile([64, S], bf16, name="tcd", tag="scr")
                    nc.vector.tensor_mul(tmp_cos[0:64, :], t[64:128, :],
                                         cos4[64:128, :])
                    nc.vector.tensor_add(t[64:128, :], tmp_cos[0:64, :],
                                         tmp_sin[0:64, :])

            if phases == 2 or phases == 3:
                for j in range(NP):
                    for sc in (2 * j, 2 * j + 1):
                        ps = ps1_p.tile([128, 512], f32, name="ps2v",
                                        tag="ps1")
                        for l in range(2):
                            nc.tensor.matmul(
                                ps[:],
                                latn[2 + l][sc // 8][:, (sc % 8) * 128:
                                                     (sc % 8 + 1) * 128],
                                wv_sb[l][:],
                                start=(l == 0), stop=(l == 1),
                            )
                        nc.vector.tensor_copy(
                            v_nat2[j][:, (sc % 2) * 512:(sc % 2 + 1) * 512],
                            ps[:])
                for mt in [0, 2, 1, 3]:
                    for n in range(NQ):
                        nsl = slice(n * 512, (n + 1) * 512)
                        ps = ps1_p.tile([128, 512], f32, name="ps2",
                                        tag="ps1")
                        for l in range(2):
                            nc.tensor.matmul(
                                ps[:],
                                wqk_sb[l][:, mt * 128:(mt + 1) * 128],
                                latn[l][n // 2][:, (n % 2) * 512:
                                                (n % 2 + 1) * 512],
                                start=(l == 0), stop=(l == 1),
                            )
                        nc.vector.tensor_copy(qkT[[0, 1, 4, 5][mt]][:, nsl],
                                              ps[:])
                rope_tiles_dbg([2, 6, 3, 7, 0, 4, 1, 5])
                for i, t in enumerate(qkT):
                    nc.sync.dma_start(out_d.ap()[i * 128:(i + 1) * 128, :],
                                      t[:])
                if phases == 3:
                    for j in range(NP):
                        nc.sync.dma_start(
                            out_d.ap()[1024 + j * 128:1024 + (j + 1) * 128,
                                       0:1024],
                            v_nat2[j][:])

            # free stage-1 pools now: their release only waits stage-1 ops
            xt_cm.__exit__(None, None, None)
            wbig_cm.__exit__(None, None, None)
            ps1_cm.__exit__(None, None, None)

            # ---------------- attention + v-dec + output projection --------
            if phases >= 4:
              with tc.tile_pool(name="wo", bufs=1) as wo_p, \
                 tc.tile_pool(name="exp", bufs=9) as exp_p, \
                 tc.tile_pool(name="den1", bufs=5) as den1_p, \
                 tc.tile_pool(name="den2", bufs=3) as den2_p, \
                 tc.tile_pool(name="den3", bufs=2) as den3_p, \
                 tc.tile_pool(name="acc", bufs=1) as acc_p, \
                 tc.tile_pool(name="ctx", bufs=9) as ctx_p, \
                 tc.tile_pool(name="rden", bufs=1) as rden_p, \
                 tc.tile_pool(name="stage", bufs=3) as stage_p, \
                 tc.tile_pool(name="ps_s", bufs=2, space="PSUM") as ps_s_p, \
                 tc.tile_pool(name="ps_c", bufs=2, space="PSUM") as ps_c_p, \
                 tc.tile_pool(name="ps_o", bufs=2, space="PSUM") as ps_o_p:
                # wo loads on the Sync queue: the ACT queue must reach the
                # first exps with no DMA issues in front of them
                wo_sb = []
                for kk in range(4):
                    w_t = wo_p.tile([128, D], bf16, name=f"wo{kk}",
                                    tag=f"wo{kk}")
                    nc.sync.dma_start(w_t[:],
                                      w_o.ap()[kk * 128:(kk + 1) * 128, :])
                    wo_sb.append(w_t)

                # k-head columns 1024:2048 (key positions, needed from
                # group 0's pair 4): rope the (6,7) pair now; the q-head
                # (2,3) second halves defer until their m=4/5 chains run
                rope_pair(6, 7, 1024, S)

                # v pair: kc 2j,2j+1 -> one [128,1024] psum -> one copy
                def emit_v_pair(j, copy_eng):
                    ps = ps_s_p.tile([128, 1024], f32, name="pss", tag="pss")
                    for half, sc in enumerate((2 * j, 2 * j + 1)):
                        hsl = slice(half * 512, (half + 1) * 512)
                        for l in range(2):
                            nc.tensor.matmul(
                                ps[:, hsl],
                                latn[2 + l][sc // 8][:, (sc % 8) * 128:
                                                     (sc % 8 + 1) * 128],
                                wv_sb[l][:],
                                start=(l == 0), stop=(l == 1),
                            )
                    if copy_eng is nc.scalar:
                        copy_eng.copy(v_nat2[j][:], ps[:])
                    else:
                        copy_eng.tensor_copy(v_nat2[j][:], ps[:])

                # serial v pairs 0..5 (kc 0..11): copies alternate
                # ACT/DVE so the ps_s rotation (and with it group 0's
                # first scores) isn't gated on a single serialized engine
                for j in range(6):
                    emit_v_pair(j, nc.scalar if j % 2 == 0 else nc.vector)

                # ---- weave queue: PE filler units ----
                weave_q = []
                tail_mode = [False]

                def weave(n=1):
                    for _ in range(n):
                        if weave_q:
                            weave_q.pop(0)()

                def mk_v_unit(j):
                    def emit():
                        emit_v_pair(j, nc.vector)
                    return emit

                def mk_dec_unit(mt, n):
                    def emit():
                        nsl = slice(n * 512, (n + 1) * 512)
                        ps = ps_o_p.tile([128, 512], f32, name="pso",
                                         tag="pso")
                        for l in range(2):
                            nc.tensor.matmul(
                                ps[:],
                                wqk_sb[l][:, mt * 128:(mt + 1) * 128],
                                latn[l][n // 2][:, (n % 2) * 512:
                                                (n % 2 + 1) * 512],
                                start=(l == 0), stop=(l == 1),
                            )
                        nc.vector.tensor_copy(qkT[[0, 1, 4, 5][mt]][:, nsl],
                                              ps[:])
                    return emit

                def mk_chain_unit(m, sub):
                    # deferred stage-1: qkT[2 or 3] columns 1024:2048,
                    # one 16-deep chain (3.4us of PE filler)
                    def emit():
                        w_sb = w4_sb if m == 4 else w5_sb
                        ps = ps_o_p.tile([128, 512], f32, name="psx",
                                         tag="pso")
                        for k in range(KD):
                            nc.tensor.matmul(
                                ps[:],
                                w_sb[:, k * 128:(k + 1) * 128],
                                xt2_tiles[k][:, sub * 512:(sub + 1) * 512],
                                start=(k == 0), stop=(k == KD - 1),
                            )
                        dst = qkT[2 if m == 4 else 3]
                        nc.vector.tensor_copy(
                            dst[:, 1024 + sub * 512:1024 + (sub + 1) * 512],
                            ps[:])
                    return emit

                def mk_out_unit(qc, ctx_by_head, m):
                    def emit():
                        qsl = slice(qc * 512, (qc + 1) * 512)
                        ps_o = ps_o_p.tile([128, 512], f32, name="pso",
                                           tag="pso")
                        for kk in range(4):
                            nc.tensor.matmul(
                                ps_o[:],
                                wo_sb[kk][:, m * 128:(m + 1) * 128],
                                ctx_by_head[kk][:],
                                start=(kk == 0), stop=(kk == 3),
                            )
                        st = stage_p.tile([128, 512], bf16, name="stg",
                                          tag="stage")
                        if tail_mode[0]:
                            nc.scalar.copy(st[:], ps_o[:])
                        else:
                            nc.vector.tensor_copy(st[:], ps_o[:])
                        nc.sync.dma_start(
                            out_d.ap()[m * 128:(m + 1) * 128, qsl], st[:])
                    return emit

                # v pairs 6,7 (kc 12..15) pop in group 0 long before
                # those avs; dec pairs at cap 6: qkT[0] done in g0,
                # qkT[4] in g1 -> rope [0,4] after g1 (used g4);
                # qkT[1]/qkT[5] done in g2 -> rope [1,5] after g2 (used
                # g5).
                weave_q.append(mk_v_unit(6))
                weave_q.append(mk_v_unit(7))
                for mt in [0, 2, 1, 3]:
                    for n in range(NQ):
                        weave_q.append(mk_dec_unit(mt, n))
                for m in (4, 5):
                    for sub in range(2):
                        weave_q.append(mk_chain_unit(m, sub))

                NSLOT = 10

                def emit_group(qc, h, wcap):
                    # one (q-chunk, head) attention block: 8 kc-pair slots,
                    # avs lag one pair, weave pops spread evenly across the
                    # NSLOT slots (pre-slot, 8 pair slots, post-tree slot).
                    qsl = slice(qc * 512, (qc + 1) * 512)
                    ps_ctx = ps_c_p.tile([128, 512], f32, name="psc",
                                         tag="psc")
                    exps2 = []
                    dlvl1 = []
                    dlvl2 = []

                    def weave_slot(sj):
                        # ceil-spread: first pop lands at slot 0 so the
                        # group never leads with 4 bare scores matmuls
                        pops = (-((-wcap * (sj + 1)) // NSLOT)
                                - -((-wcap * sj) // NSLOT))
                        for _ in range(pops):
                            if weave_q:
                                weave_q.pop(0)()

                    def exp_half(p, half):
                        return exps2[p][:, half * 512:(half + 1) * 512]

                    def emit_scores_pair(p):
                        ps_s = ps_s_p.tile([128, 1024], f32, name="pss",
                                           tag="pss")
                        for half, kc in enumerate((2 * p, 2 * p + 1)):
                            nc.tensor.matmul(
                                ps_s[:, half * 512:(half + 1) * 512],
                                qkT[4 + h][:, kc * 128:(kc + 1) * 128],
                                qkT[h][:, qsl],
                                start=True, stop=True,
                            )
                        expT = exp_p.tile([128, 1024], bf16, name="expT",
                                          tag="exp")
                        nc.scalar.activation(
                            expT[:], ps_s[:],
                            mybir.ActivationFunctionType.Exp, scale=SCALE)
                        exps2.append(expT)
                        # den tree in full-width bf16 ops (DVE fixed cost
                        # ~270-400ns/op dominates narrow adds)
                        if p % 2 == 1:
                            d = den1_p.tile([128, 1024], bf16, name="d1",
                                            tag="d1")
                            nc.vector.tensor_add(d[:], exps2[p - 1][:],
                                                 exps2[p][:])
                            dlvl1.append(d)
                            if p % 4 == 3:
                                d2 = den2_p.tile([128, 1024], bf16,
                                                 name="d2", tag="d2")
                                nc.vector.tensor_add(
                                    d2[:], dlvl1[p // 4 * 2][:],
                                    dlvl1[p // 4 * 2 + 1][:])
                                dlvl2.append(d2)

                    def emit_av(kc):
                        nc.tensor.matmul(
                            ps_ctx[:],
                            v_slice(kc, h),
                            exp_half(kc // 2, kc % 2),
                            start=(kc == 0), stop=(kc == NK - 1),
                        )

                    weave_slot(0)
                    for p in range(NP):
                        emit_scores_pair(p)
                        if p >= 2:
                            emit_av(2 * p - 4)
                            emit_av(2 * p - 3)
                        weave_slot(p + 1)
                    emit_av(NK - 4)
                    emit_av(NK - 3)
                    # eager den tree finale (wide bf16, then fold halves)
                    d3 = den3_p.tile([128, 1024], bf16, name="d3", tag="d3")
                    nc.vector.tensor_add(d3[:], dlvl2[0][:], dlvl2[1][:])
                    acc = acc_p.tile([128, 512], bf16, name="acc", tag="acc")
                    nc.vector.tensor_add(acc[:], d3[:, 0:512],
                                         d3[:, 512:1024])
                    emit_av(NK - 2)
                    emit_av(NK - 1)
                    weave_slot(NSLOT - 1)
                    ps_den = ps_o_p.tile([128, 512], f32, name="psd",
                                         tag="pso")
                    nc.tensor.matmul(ps_den[:], ones_bf[:], acc[:],
                                     start=True, stop=True)
                    rden = rden_p.tile([128, 512], f32, name="rden",
                                       tag="rden")
                    nc.vector.reciprocal_approx_fast(rden[:], ps_den[:])
                    c_t = ctx_p.tile([128, 512], bf16, name="ctxt",
                                     tag="ctx")
                    nc.vector.tensor_mul(c_t[:], ps_ctx[:], rden[:])
                    if phases == 5:
                        r0 = (qc * 4 + h) * 128
                        nc.sync.dma_start(out_d.ap()[r0:r0 + 128, 0:512],
                                          c_t[:])
                    return c_t

                # x-projection heads first; dec-head groups after their
                # woven decompression + rope.
                order = [(0, 2), (0, 3), (1, 2), (0, 0), (0, 1),
                         (1, 3), (1, 0), (1, 1),
                         (2, 2), (2, 3), (2, 0), (2, 1),
                         (3, 2), (3, 3), (3, 0), (3, 1)]
                caps = [6, 6, 6, 2, 2,
                        6, 5, 5,
                        4, 4, 4, 4,
                        4, 4, 4, 4]
                ctxs = {}
                for gi, (qc, h) in enumerate(order):
                    ctxs.setdefault(qc, {})[h] = emit_group(qc, h, caps[gi])
                    if gi == 1:
                        rope_pair(0, 4)
                    if gi == 2:
                        rope_pair(1, 5)
                    if gi == 4:
                        # q-head columns 1024:2048 (deferred chains done in
                        # g3/g4); first consumer is (2,2) at g8
                        rope_pair(2, 3, 1024, S)
                    if len(ctxs[qc]) == 4:
                        dct = ctxs.pop(qc)
                        for m in range(16):
                            weave_q.append(mk_out_unit(qc, dct, m))
                tail_mode[0] = True
                while weave_q:
                    weave()
            w45_cm.__exit__(None, None, None)
            xt2_cm.__exit__(None, None, None)
            lat_cm.__exit__(None, None, None)
            wdec_cm.__exit__(None, None, None)
            scr_cm.__exit__(None, None, None)
            swp_cm.__exit__(None, None, None)
            consts_cm.__exit__(None, None, None)

    nc.compile()
    return nc


def _get_program():
    if "nc" not in _prog_cache:
        _prog_cache["nc"] = _build_program()
    return _prog_cache["nc"]


def _host_shards(x, W_comp, W_q_dec, W_k_dec, W_v_dec, W_rope_q, W_rope_k,
                 W_out):
    import ml_dtypes
    bf16 = ml_dtypes.bfloat16

    inv = 1.0 / (10000.0 ** (np.arange(0, RD, 2, dtype=np.float32) / RD))
    ang = np.arange(S, dtype=np.float32)[:, None] * inv[None, :]     # [S, 32]
    cosT = np.cos(ang).T.astype(np.float32)                          # [32, S]
    sinT = np.sin(ang).T.astype(np.float32)
    cos4 = np.ascontiguousarray(np.tile(cosT, (4, 1))).astype(bf16)  # [128,S]
    sin4w = np.ascontiguousarray(np.tile(
        np.concatenate([-sinT, sinT], axis=0), (2, 1))).astype(bf16)  # [128,S]

    in_maps = []
    for c in range(NC):
        b, hg = divmod(c, 4)
        xTb = np.ascontiguousarray(x[b].T.astype(bf16))
        w_big = np.ascontiguousarray(np.concatenate(
            [W_comp,
             W_rope_q[:, hg * 256:(hg + 1) * 256],
             W_rope_k[:, hg * 256:(hg + 1) * 256]], axis=1).astype(bf16))
        w_qk = np.ascontiguousarray(np.concatenate(
            [W_q_dec[:, hg * 256:(hg + 1) * 256],
             W_k_dec[:, hg * 256:(hg + 1) * 256]], axis=1).astype(bf16))
        w_v = np.ascontiguousarray(np.concatenate(
            [W_v_dec[:, hg * 256:(hg + 1) * 256],
             W_v_dec[:, 1024 + hg * 256:1024 + (hg + 1) * 256]],
            axis=1).astype(bf16))
        w_o = np.ascontiguousarray(np.concatenate(
            [W_out[hg * 256:(hg + 1) * 256, :],
             W_out[1024 + hg * 256:1024 + (hg + 1) * 256, :]],
            axis=0).astype(bf16))
        w4 = np.ascontiguousarray(np.concatenate(
            [w_big[k * 128:(k + 1) * 128, 512:640] for k in range(16)],
            axis=1))
        w5 = np.ascontiguousarray(np.concatenate(
            [w_big[k * 128:(k + 1) * 128, 640:768] for k in range(16)],
            axis=1))
        in_maps.append({
            "xT": xTb, "w_big": w_big, "w_qk": w_qk, "w_v": w_v, "w_o": w_o,
            "w4": w4, "w5": w5, "cos4": cos4, "sin4w": sin4w,
        })
    return in_maps


def kernel(x, W_comp, W_q_dec, W_k_dec, W_v_dec, W_rope_q, W_rope_k, W_out,
           _trace=False):
    from concourse import bass_utils

    x = np.asarray(x, np.float32)
    args = [np.asarray(a, np.float32)
            for a in (W_comp, W_q_dec, W_k_dec, W_v_dec,
                      W_rope_q, W_rope_k, W_out)]
    in_maps = _host_shards(x, *args)
    nc = _get_program()
    res = bass_utils.run_bass_kernel_spmd(
        nc, in_maps, core_ids=list(range(NC)), trace=_trace)
    out = np.zeros((B, S, D), np.float32)
    for c in range(NC):
        b = c // 4
        out[b] += res.results[c]["out"].astype(np.float32).T
    if _trace:
        kernel.last_exec_ns = res.exec_time_ns
    return out


# revision 23
# speedup vs baseline: 1.0453x; 1.0354x over previous
"""Multi-Head Latent Attention (MLA) Trainium2 kernel — v5.

Problem (hardcoded): B=2, S=2048, D_MODEL=2048, H=16, HEAD_DIM=128,
D_LATENT=512 (D_QK=256 / D_V=256), ROPE_DIM=64, fp32 in/out.

Reference semantics: q = concat([q_no_rope(1024), q_rope(1024)]).reshape(16
heads x 128), so heads 0-7 take both 64-dim halves from the latent
decompression and heads 8-15 take both halves from the rope projection of x;
RoPE rotates dims 64:128 of every head.

Sharding: 8 cores = 2 batches x 4 head-groups; core (b, hg) owns heads
[2hg, 2hg+1, 8+2hg, 8+2hg+1] (2 decompression + 2 rope-projection heads),
computes the shared latent for its batch redundantly, and produces a partial
output projection (its heads' rows of W_out), transposed [e, q]. The host
sums the 4 partials per batch (in f32; device emits bf16 partials).

v5 design notes (measured on HW: v2=351.5us, v3=360.8, v4=370.8):
  - PE matmuls stream at 215ns issue-to-issue; ~292us of matmul issue is
    the floor. ACT exp on [128,512] measured ~716ns (290 fixed + 426
    compute at 1.2GHz), so a bare attention group is ACT-bound: 16 exps =
    11.4us vs 7.1us of PE. v2-v4 all paid distributed PE stalls for this.
  - exp PAIRING: scores for kc pairs write the two 512-wide halves of one
    [128,1024] psum tile (two adjacent banks) and ONE exp activation
    covers both: 8x1143ns = 9.1us ACT per group. With >=2us of woven PE
    filler per group the whole attention phase is PE-bound.
  - v decompression also runs in [128,1024] pairs (v_nat2[j] holds kc=2j
    and 2j+1 side by side; av slices columns), 6 pairs serial + 2 as weave
    units inside group 0.
  - single weave queue of PE filler units (v pairs, the 16 q/k
    decompression pairs in groups 0-3 meeting the rope deadlines, then
    each q-chunk's out-proj m-tiles), popped at EVENLY SPACED slots across
    each group's pair stream (v4 front-loaded the budget and left group
    tails ACT-bound).
  - rope processes head PAIRS with full-width [128,S] DVE ops + DMA
    write-back (3.6us DVE per pair vs 7.2 for [64,S] ops).
  - softmax denominator: eager DVE add-tree (lvl1 halves of each exp pair,
    lvl3 right after the last lvl2), bf16 ones-colsum matmul at group end.
  - barrier-free phase transition: stage-1 pools freed immediately (their
    release only waits stage-1 ops; the last psum group's copies split
    ACT/DVE to halve the drain), attention pools allocated, v pairs run
    inside them.
  - all attention-phase psum->SBUF copies on DVE; ACT does exps only; wo
    loads on the Sync queue so they never delay the first exps.
"""

import math

import numpy as np

B = 2
S = 2048
D = 2048
H4 = 4            # heads per core
HD = 128          # head dim
DL = 512          # d_latent
DQK = 256
RD = 64           # rope dim
NC = 8            # cores

SCALE = 1.0 / math.sqrt(HD)

_prog_cache = {}


def _build_program(phases=4):
    import concourse.tile as tile
    from concourse import bacc, mybir

    bf16 = mybir.dt.bfloat16
    f32 = mybir.dt.float32

    nc = bacc.Bacc("TRN2", target_bir_lowering=False, debug=False, num_devices=1)

    xT = nc.dram_tensor("xT", [D, S], bf16, kind="ExternalInput")
    w_big = nc.dram_tensor("w_big", [D, 1024], bf16, kind="ExternalInput")
    w_qk = nc.dram_tensor("w_qk", [DQK, 512], bf16, kind="ExternalInput")
    w_v = nc.dram_tensor("w_v", [DQK, 512], bf16, kind="ExternalInput")
    w_o = nc.dram_tensor("w_o", [DL, D], bf16, kind="ExternalInput")
    w4_d = nc.dram_tensor("w4", [128, 2048], bf16, kind="ExternalInput")
    w5_d = nc.dram_tensor("w5", [128, 2048], bf16, kind="ExternalInput")
    cos4_d = nc.dram_tensor("cos4", [128, S], bf16, kind="ExternalInput")
    sin4w_d = nc.dram_tensor("sin4w", [128, S], bf16, kind="ExternalInput")
    out_d = nc.dram_tensor("out", [D, S], bf16, kind="ExternalOutput")

    NQ = S // 512    # 4 q chunks of 512 (attention)
    NK = S // 128    # 16 k/seq chunks of 128
    NP = NK // 2     # 8 kc pairs per group
    KD = D // 128    # 16 contraction chunks for stage 1
    N4 = S // 1024   # 2 wide n-chunks of 1024 (stage1)

    with tile.TileContext(nc, pool_alloc_mode="queue") as tc:
        import contextlib

        with contextlib.ExitStack() as ctx:
            # persistent pools (live to end of program, LIFO via ExitStack)
            ones_p = ctx.enter_context(tc.tile_pool(name="onesp", bufs=1))
            qk_p = ctx.enter_context(tc.tile_pool(name="qk", bufs=1))
            v_p = ctx.enter_context(tc.tile_pool(name="vp", bufs=1))

            ones_f32 = ones_p.tile([128, 128], f32)
            nc.gpsimd.memset(ones_f32[:], 1.0)
            ones_bf = ones_p.tile([128, 128], bf16)
            nc.vector.tensor_copy(ones_bf[:], ones_f32[:])
            # per-head assembled q/k: rows 0:64 nr dims, 64:128 roped dims
            # qkT[0..3] = q heads 0..3, qkT[4..7] = k heads 0..3
            qkT = [qk_p.tile([128, S], bf16, name=f"qkT{i}", tag=f"qk{i}")
                   for i in range(8)]
            # v pairs: v_nat2[j] cols 0:512 = kc 2j, cols 512:1024 = kc 2j+1
            v_nat2 = [v_p.tile([128, 1024], bf16, name=f"v{j}", tag=f"v{j}")
                      for j in range(NP)]

            def v_slice(kc, h):
                base = (kc % 2) * 512 + h * 128
                return v_nat2[kc // 2][:, base:base + 128]

            consts_cm = tc.tile_pool(name="consts", bufs=1)
            consts = consts_cm.__enter__()
            swp_cm = tc.tile_pool(name="swpA", bufs=2)
            swp_p = swp_cm.__enter__()
            scr_cm = tc.tile_pool(name="scrA", bufs=3)
            scr_p = scr_cm.__enter__()

            # wdec/lat outlive the stage-1 pools (the weave dec units read
            # them mid-attention) — right-side stack.
            wdec_cm = tc.tile_pool(name="wdec", bufs=1, side="right")
            wdec_p = wdec_cm.__enter__()
            lat_cm = tc.tile_pool(name="lat", bufs=1, side="right")
            lat_p = lat_cm.__enter__()
            xt2_cm = tc.tile_pool(name="xt2", bufs=1, side="right")
            xt2_p = xt2_cm.__enter__()
            w45_cm = tc.tile_pool(name="w45", bufs=1, side="right")
            w45_p = w45_cm.__enter__()
            # latn[l][n4]: latent rows l*128:(l+1)*128, cols n4*1024:+1024
            latn = [[lat_p.tile([128, 1024], bf16, name=f"latT{i}_{n}",
                                tag=f"lat{i}_{n}") for n in range(N4)]
                    for i in range(4)]

            # ---------------- stage 1: bigT = w_big^T @ xT -----------------
            ps1_cm = tc.tile_pool(name="ps1", bufs=8, space="PSUM")
            ps1_p = ps1_cm.__enter__()
            wbig_cm = tc.tile_pool(name="wbig", bufs=1)
            wbig_p = wbig_cm.__enter__()
            xt_cm = tc.tile_pool(name="xt", bufs=18)
            xt_p = xt_cm.__enter__()

            wbig_sb = [wbig_p.tile([128, 1024], bf16, name=f"wb{k}",
                                   tag=f"wb{k}") for k in range(KD)]
            # ---------------- rope helper (head pair, full-width DVE) ------
            # roped rows t[64:128] = raw*cos + swap32(raw)*sin for two head
            # tiles at once: swapped and raw halves of BOTH tiles stacked
            # into [128,S] scratch (DMA), three full-width DVE ops, two
            # DMAs write the rows back.
            def rope_pair(i, j, c0=0, c1=S):
                ti, tj = qkT[i], qkT[j]
                w = c1 - c0
                sw = swp_p.tile([128, S], bf16, name=f"sw{i}_{j}", tag="sw")
                nc.sync.dma_start(sw[0:32, 0:w], ti[96:128, c0:c1])
                nc.sync.dma_start(sw[32:64, 0:w], ti[64:96, c0:c1])
                nc.sync.dma_start(sw[64:96, 0:w], tj[96:128, c0:c1])
                nc.sync.dma_start(sw[96:128, 0:w], tj[64:96, c0:c1])
                raw = scr_p.tile([128, S], bf16, name="raw", tag="scr")
                nc.sync.dma_start(raw[0:64, 0:w], ti[64:128, c0:c1])
                nc.sync.dma_start(raw[64:128, 0:w], tj[64:128, c0:c1])
                tsin = scr_p.tile([128, S], bf16, name="tsin", tag="scr")
                nc.vector.tensor_mul(tsin[:, 0:w], sw[:, 0:w],
                                     sin4w[:, c0:c1])
                res = scr_p.tile([128, S], bf16, name="res", tag="scr")
                nc.vector.tensor_mul(res[:, 0:w], raw[:, 0:w],
                                     cos4[:, c0:c1])
                nc.vector.tensor_add(res[:, 0:w], res[:, 0:w],
                                     tsin[:, 0:w])
                nc.sync.dma_start(ti[64:128, c0:c1], res[0:64, 0:w])
                nc.sync.dma_start(tj[64:128, c0:c1], res[64:128, 0:w])


            rope_early = [False]
            for n4 in range(N4):
                if n4 == 1 and phases >= 4:
                    rope_pair(2, 6, 0, 1024)
                    rope_pair(3, 7, 0, 1024)
                    rope_early[0] = True
                xts = []
                for k in range(KD):
                    if n4 == 0:
                        nc.scalar.dma_start(wbig_sb[k][:],
                                            w_big.ap()[k * 128:(k + 1) * 128, :])
                    if n4 == 0 or phases < 4:
                        x_t = xt_p.tile([128, 1024], bf16, name="xt",
                                        tag="xt")
                    else:
                        # n4=1 tiles outlive stage 1: the deferred m=4/5
                        # chains read them mid-attention
                        x_t = xt2_p.tile([128, 1024], bf16, name=f"x2_{k}",
                                         tag=f"x2_{k}")
                    nc.sync.dma_start(
                        x_t[:], xT.ap()[k * 128:(k + 1) * 128,
                                        n4 * 1024:(n4 + 1) * 1024])
                    xts.append(x_t)
                if n4 == 1 and phases >= 4:
                    xt2_tiles = xts
                if n4 == 0:
                    wqk_sb = []
                    for l in range(2):
                        w_t = wdec_p.tile([128, 512], bf16, name=f"wqk{l}",
                                          tag=f"wqk{l}")
                        nc.scalar.dma_start(w_t[:],
                                            w_qk.ap()[l * 128:(l + 1) * 128, :])
                        wqk_sb.append(w_t)
                    wv_sb = []
                    for l in range(2):
                        w_t = wdec_p.tile([128, 512], bf16, name=f"wv{l}",
                                          tag=f"wv{l}")
                        nc.scalar.dma_start(w_t[:],
                                            w_v.ap()[l * 128:(l + 1) * 128, :])
                        wv_sb.append(w_t)
                    cos4 = consts.tile([128, S], bf16)
                    nc.scalar.dma_start(cos4[:], cos4_d.ap()[:])
                    sin4w = consts.tile([128, S], bf16)
                    nc.scalar.dma_start(sin4w[:], sin4w_d.ap()[:])
                    w4_sb = w45_p.tile([128, 2048], bf16, name="w4",
                                       tag="w4")
                    nc.scalar.dma_start(w4_sb[:], w4_d.ap()[:])
                    w5_sb = w45_p.tile([128, 2048], bf16, name="w5",
                                       tag="w5")
                    nc.scalar.dma_start(w5_sb[:], w5_d.ap()[:])
                for sub in range(2):
                    last_grp = (n4 == N4 - 1 and sub == 1)
                    mlist = (list(range(8)) if (n4 == 0 or phases < 4)
                             else [0, 1, 2, 3, 6, 7])
                    psums = [ps1_p.tile([128, 512], f32, name=f"ps1_{m}",
                                        tag="ps1") for m in mlist]
                    for k in range(KD):
                        for mi, m in enumerate(mlist):
                            nc.tensor.matmul(
                                psums[mi][:],
                                wbig_sb[k][:, m * 128:(m + 1) * 128],
                                xts[k][:, sub * 512:(sub + 1) * 512],
                                start=(k == 0),
                                stop=(k == KD - 1),
                            )
                    lsl = slice(sub * 512, (sub + 1) * 512)
                    nsl = slice(n4 * 1024 + sub * 512,
                                n4 * 1024 + (sub + 1) * 512)
                    for mi, m in enumerate(mlist):
                        dst = (latn[m][n4][:, lsl] if m < 4
                               else qkT[[2, 3, 6, 7][m - 4]][:, nsl])
                        if last_grp and m % 2 == 1:
                            # split the final drain ACT/DVE so the psum
                            # banks recycle fast for the v pairs
                            nc.scalar.copy(dst, psums[mi][:])
                        else:
                            nc.vector.tensor_copy(dst, psums[mi][:])

            # exp-table warm-up on the scalar queue behind the DMA issues
            warm = ones_p.tile([128, 1], f32)
            nc.scalar.activation(warm[:], ones_f32[:, 0:1],
                                 mybir.ActivationFunctionType.Exp)

            if phases == 1:
                for i in range(4):
                    for n in range(N4):
                        nc.sync.dma_start(
                            out_d.ap()[i * 128:(i + 1) * 128,
                                       n * 1024:(n + 1) * 1024],
                            latn[i][n][:])
                for i, t in enumerate(qkT):
                    nc.sync.dma_start(
                        out_d.ap()[512 + i * 128:512 + (i + 1) * 128, :], t[:])

            # debug-path rope (serial, in-place)
            def rope_tiles_dbg(idxs):
                for i in idxs:
                    t = qkT[i]
                    sw = swp_p.tile([64, S], bf16, name=f"swd{i}", tag="sw")
                    nc.sync.dma_start(sw[0:32, :], t[96:128, :])
                    nc.sync.dma_start(sw[32:64, :], t[64:96, :])
                    tmp_sin = scr_p.tile([64, S], bf16, name="tsd", tag="scr")
                    nc.vector.tensor_mul(tmp_sin[0:64, :], sw[0:64, :],
                                         sin4w[0:64, :])
                    tmp_cos = scr_p.tile([64, S], bf16, name="tcd", tag="scr")
                    nc.vector.tensor_mul(tmp_cos[0:64, :], t[64:128, :],
                                         cos4[64:128, :])
                    nc.vector.tensor_add(t[64:128, :], tmp_cos[0:64, :],
                                         tmp_sin[0:64, :])

            if phases == 2 or phases == 3:
                for j in range(NP):
                    for sc in (2 * j, 2 * j + 1):
                        ps = ps1_p.tile([128, 512], f32, name="ps2v",
                                        tag="ps1")
                        for l in range(2):
                            nc.tensor.matmul(
                                ps[:],
                                latn[2 + l][sc // 8][:, (sc % 8) * 128:
                                                     (sc % 8 + 1) * 128],
                                wv_sb[l][:],
                                start=(l == 0), stop=(l == 1),
                            )
                        nc.vector.tensor_copy(
                            v_nat2[j][:, (sc % 2) * 512:(sc % 2 + 1) * 512],
                            ps[:])
                for mt in [0, 2, 1, 3]:
                    for n in range(NQ):
                        nsl = slice(n * 512, (n + 1) * 512)
                        ps = ps1_p.tile([128, 512], f32, name="ps2",
                                        tag="ps1")
                        for l in range(2):
                            nc.tensor.matmul(
                                ps[:],
                                wqk_sb[l][:, mt * 128:(mt + 1) * 128],
                                latn[l][n // 2][:, (n % 2) * 512:
                                                (n % 2 + 1) * 512],
                                start=(l == 0), stop=(l == 1),
                            )
                        nc.vector.tensor_copy(qkT[[0, 1, 4, 5][mt]][:, nsl],
                                              ps[:])
                rope_tiles_dbg([2, 6, 3, 7, 0, 4, 1, 5])
                for i, t in enumerate(qkT):
                    nc.sync.dma_start(out_d.ap()[i * 128:(i + 1) * 128, :],
                                      t[:])
                if phases == 3:
                    for j in range(NP):
                        nc.sync.dma_start(
                            out_d.ap()[1024 + j * 128:1024 + (j + 1) * 128,
                                       0:1024],
                            v_nat2[j][:])

            # free stage-1 pools now: their release only waits stage-1 ops
            xt_cm.__exit__(None, None, None)
            wbig_cm.__exit__(None, None, None)
            ps1_cm.__exit__(None, None, None)

            # ---------------- attention + v-dec + output projection --------
            if phases >= 4:
              with tc.tile_pool(name="wo", bufs=1) as wo_p, \
                 tc.tile_pool(name="exp", bufs=10) as exp_p, \
                 tc.tile_pool(name="den1", bufs=5) as den1_p, \
                 tc.tile_pool(name="den2", bufs=3) as den2_p, \
                 tc.tile_pool(name="den3", bufs=1) as den3_p, \
                 tc.tile_pool(name="acc", bufs=1) as acc_p, \
                 tc.tile_pool(name="ctx", bufs=9) as ctx_p, \
                 tc.tile_pool(name="rden", bufs=1) as rden_p, \
                 tc.tile_pool(name="stage", bufs=3) as stage_p, \
                 tc.tile_pool(name="ps_s", bufs=2, space="PSUM") as ps_s_p, \
                 tc.tile_pool(name="ps_c", bufs=2, space="PSUM") as ps_c_p, \
                 tc.tile_pool(name="ps_o", bufs=2, space="PSUM") as ps_o_p:
                # wo loads on the Sync queue: the ACT queue must reach the
                # first exps with no DMA issues in front of them
                wo_sb = []
                for kk in range(4):
                    w_t = wo_p.tile([128, D], bf16, name=f"wo{kk}",
                                    tag=f"wo{kk}")
                    nc.sync.dma_start(w_t[:],
                                      w_o.ap()[kk * 128:(kk + 1) * 128, :])
                    wo_sb.append(w_t)

                # k-head columns 1024:2048 (key positions, needed from
                # group 0's pair 4): rope the (6,7) pair now; the q-head
                # (2,3) second halves defer until their m=4/5 chains run
                rope_pair(6, 7, 1024, S)

                # v pair: kc 2j,2j+1 -> one [128,1024] psum -> one copy
                def emit_v_pair(j, copy_eng):
                    ps = ps_s_p.tile([128, 1024], f32, name="pss", tag="pss")
                    for half, sc in enumerate((2 * j, 2 * j + 1)):
                        hsl = slice(half * 512, (half + 1) * 512)
                        for l in range(2):
                            nc.tensor.matmul(
                                ps[:, hsl],
                                latn[2 + l][sc // 8][:, (sc % 8) * 128:
                                                     (sc % 8 + 1) * 128],
                                wv_sb[l][:],
                                start=(l == 0), stop=(l == 1),
                            )
                    if copy_eng is nc.scalar:
                        copy_eng.copy(v_nat2[j][:], ps[:])
                    else:
                        copy_eng.tensor_copy(v_nat2[j][:], ps[:])

                # serial v pairs 0..5 (kc 0..11): copies alternate
                # ACT/DVE so the ps_s rotation (and with it group 0's
                # first scores) isn't gated on a single serialized engine
                for j in range(6):
                    emit_v_pair(j, nc.scalar if j % 2 == 0 else nc.vector)

                # ---- weave queue: PE filler units ----
                weave_q = []
                tail_mode = [False]

                def weave(n=1):
                    for _ in range(n):
                        if weave_q:
                            weave_q.pop(0)()

                def mk_v_unit(j):
                    def emit():
                        emit_v_pair(j, nc.vector)
                    return emit

                def mk_dec_unit(mt, n):
                    def emit():
                        nsl = slice(n * 512, (n + 1) * 512)
                        ps = ps_o_p.tile([128, 512], f32, name="pso",
                                         tag="pso")
                        for l in range(2):
                            nc.tensor.matmul(
                                ps[:],
                                wqk_sb[l][:, mt * 128:(mt + 1) * 128],
                                latn[l][n // 2][:, (n % 2) * 512:
                                                (n % 2 + 1) * 512],
                                start=(l == 0), stop=(l == 1),
                            )
                        nc.vector.tensor_copy(qkT[[0, 1, 4, 5][mt]][:, nsl],
                                              ps[:])
                    return emit

                def mk_chain_unit(m, sub):
                    # deferred stage-1: qkT[2 or 3] columns 1024:2048,
                    # one 16-deep chain (3.4us of PE filler)
                    def emit():
                        w_sb = w4_sb if m == 4 else w5_sb
                        ps = ps_o_p.tile([128, 512], f32, name="psx",
                                         tag="pso")
                        for k in range(KD):
                            nc.tensor.matmul(
                                ps[:],
                                w_sb[:, k * 128:(k + 1) * 128],
                                xt2_tiles[k][:, sub * 512:(sub + 1) * 512],
                                start=(k == 0), stop=(k == KD - 1),
                            )
                        dst = qkT[2 if m == 4 else 3]
                        nc.vector.tensor_copy(
                            dst[:, 1024 + sub * 512:1024 + (sub + 1) * 512],
                            ps[:])
                    return emit

                def mk_out_unit(qc, ctx_by_head, m):
                    def emit():
                        qsl = slice(qc * 512, (qc + 1) * 512)
                        ps_o = ps_o_p.tile([128, 512], f32, name="pso",
                                           tag="pso")
                        for kk in range(4):
                            nc.tensor.matmul(
                                ps_o[:],
                                wo_sb[kk][:, m * 128:(m + 1) * 128],
                                ctx_by_head[kk][:],
                                start=(kk == 0), stop=(kk == 3),
                            )
                        st = stage_p.tile([128, 512], bf16, name="stg",
                                          tag="stage")
                        if tail_mode[0]:
                            nc.scalar.copy(st[:], ps_o[:])
                        else:
                            nc.vector.tensor_copy(st[:], ps_o[:])
                        nc.sync.dma_start(
                            out_d.ap()[m * 128:(m + 1) * 128, qsl], st[:])
                    return emit

                # v pairs 6,7 (kc 12..15) pop in group 0 long before
                # those avs; dec pairs at cap 6: qkT[0] done in g0,
                # qkT[4] in g1 -> rope [0,4] after g1 (used g4);
                # qkT[1]/qkT[5] done in g2 -> rope [1,5] after g2 (used
                # g5).
                weave_q.append(mk_v_unit(6))
                weave_q.append(mk_v_unit(7))
                for mt in [0, 2, 1, 3]:
                    for n in range(NQ):
                        weave_q.append(mk_dec_unit(mt, n))
                for m in (4, 5):
                    for sub in range(2):
                        weave_q.append(mk_chain_unit(m, sub))

                NSLOT = 10

                def emit_group(qc, h, wcap, run_prev_tail):
                    # one (q-chunk, head) attention block: 8 kc-pair slots,
                    # avs lag one pair, weave pops spread evenly across the
                    # NSLOT slots (pre-slot, 8 pair slots, post-tree slot).
                    qsl = slice(qc * 512, (qc + 1) * 512)
                    ps_ctx = ps_c_p.tile([128, 512], f32, name="psc",
                                         tag="psc")
                    exps2 = []
                    dlvl1 = []
                    dlvl2 = []

                    def weave_slot(sj):
                        # ceil-spread: first pop lands at slot 0 so the
                        # group never leads with 4 bare scores matmuls
                        pops = (-((-wcap * (sj + 1)) // NSLOT)
                                - -((-wcap * sj) // NSLOT))
                        for _ in range(pops):
                            if weave_q:
                                weave_q.pop(0)()

                    def exp_half(p, half):
                        return exps2[p][:, half * 512:(half + 1) * 512]

                    def emit_scores_pair(p):
                        ps_s = ps_s_p.tile([128, 1024], f32, name="pss",
                                           tag="pss")
                        for half, kc in enumerate((2 * p, 2 * p + 1)):
                            nc.tensor.matmul(
                                ps_s[:, half * 512:(half + 1) * 512],
                                qkT[4 + h][:, kc * 128:(kc + 1) * 128],
                                qkT[h][:, qsl],
                                start=True, stop=True,
                            )
                        expT = exp_p.tile([128, 1024], bf16, name="expT",
                                          tag="exp")
                        nc.scalar.activation(
                            expT[:], ps_s[:],
                            mybir.ActivationFunctionType.Exp, scale=SCALE)
                        exps2.append(expT)
                        # den tree in full-width bf16 ops (DVE fixed cost
                        # ~270-400ns/op dominates narrow adds)
                        if p % 2 == 1:
                            d = den1_p.tile([128, 1024], bf16, name="d1",
                                            tag="d1")
                            nc.vector.tensor_add(d[:], exps2[p - 1][:],
                                                 exps2[p][:])
                            dlvl1.append(d)
                            if p % 4 == 3:
                                d2 = den2_p.tile([128, 1024], bf16,
                                                 name="d2", tag="d2")
                                nc.vector.tensor_add(
                                    d2[:], dlvl1[p // 4 * 2][:],
                                    dlvl1[p // 4 * 2 + 1][:])
                                dlvl2.append(d2)

                    def emit_av(kc):
                        nc.tensor.matmul(
                            ps_ctx[:],
                            v_slice(kc, h),
                            exp_half(kc // 2, kc % 2),
                            start=(kc == 0), stop=(kc == NK - 1),
                        )

                    weave_slot(0)
                    for p in range(NP):
                        emit_scores_pair(p)
                        if p >= 2:
                            emit_av(2 * p - 4)
                            emit_av(2 * p - 3)
                        weave_slot(p + 1)
                        if p == 1:
                            # previous group's tail lands here: its last
                            # exps are long done, and it covers exactly
                            # the window where this group's exp(p0) cooks
                            run_prev_tail()

                    def tail():
                        emit_av(NK - 4)
                        emit_av(NK - 3)
                        # den tree finale (wide bf16, then fold halves)
                        d3 = den3_p.tile([128, 1024], bf16, name="d3",
                                         tag="d3")
                        nc.vector.tensor_add(d3[:], dlvl2[0][:],
                                             dlvl2[1][:])
                        acc = acc_p.tile([128, 512], bf16, name="acc",
                                         tag="acc")
                        nc.vector.tensor_add(acc[:], d3[:, 0:512],
                                             d3[:, 512:1024])
                        emit_av(NK - 2)
                        emit_av(NK - 1)
                        weave_slot(NSLOT - 1)
                        ps_den = ps_o_p.tile([128, 512], f32, name="psd",
                                             tag="pso")
                        nc.tensor.matmul(ps_den[:], ones_bf[:], acc[:],
                                         start=True, stop=True)
                        rden = rden_p.tile([128, 512], f32, name="rden",
                                           tag="rden")
                        nc.vector.reciprocal_approx_fast(rden[:],
                                                         ps_den[:])
                        c_t = ctx_p.tile([128, 512], bf16, name="ctxt",
                                         tag="ctx")
                        nc.vector.tensor_mul(c_t[:], ps_ctx[:], rden[:])
                        if phases == 5:
                            r0 = (qc * 4 + h) * 128
                            nc.sync.dma_start(
                                out_d.ap()[r0:r0 + 128, 0:512], c_t[:])
                        return c_t

                    return tail

                # x-projection heads first; dec-head groups after their
                # woven decompression + rope.
                order = [(0, 2), (0, 3), (1, 2), (0, 0), (0, 1),
                         (1, 3), (1, 0), (1, 1),
                         (2, 2), (2, 3), (2, 0), (2, 1),
                         (3, 2), (3, 3), (3, 0), (3, 1)]
                caps = [6, 6, 6, 2, 2,
                        6, 5, 5,
                        4, 4, 4, 4,
                        4, 4, 4, 4]
                ctxs = {}
                pend = [None]   # (qc, h, tail_fn) of the previous group

                def flush_tail():
                    if pend[0] is not None:
                        pqc, ph, tfn = pend[0]
                        pend[0] = None
                        ctxs.setdefault(pqc, {})[ph] = tfn()
                        if len(ctxs[pqc]) == 4:
                            dct = ctxs.pop(pqc)
                            for m in range(16):
                                weave_q.append(mk_out_unit(pqc, dct, m))

                for gi, (qc, h) in enumerate(order):
                    tfn = emit_group(qc, h, caps[gi], flush_tail)
                    pend[0] = (qc, h, tfn)
                    if gi == 1:
                        rope_pair(0, 4)
                    if gi == 2:
                        rope_pair(1, 5)
                    if gi == 4:
                        # q-head columns 1024:2048 (deferred chains done in
                        # g3/g4); first consumer is (2,2) at g8
                        rope_pair(2, 3, 1024, S)
                flush_tail()
                tail_mode[0] = True
                while weave_q:
                    weave()
            w45_cm.__exit__(None, None, None)
            xt2_cm.__exit__(None, None, None)
            lat_cm.__exit__(None, None, None)
            wdec_cm.__exit__(None, None, None)
            scr_cm.__exit__(None, None, None)
            swp_cm.__exit__(None, None, None)
            consts_cm.__exit__(None, None, None)

    nc.compile()
    return nc


def _get_program():
    if "nc" not in _prog_cache:
        _prog_cache["nc"] = _build_program()
    return _prog_cache["nc"]


def _host_shards(x, W_comp, W_q_dec, W_k_dec, W_v_dec, W_rope_q, W_rope_k,
                 W_out):
    import ml_dtypes
    bf16 = ml_dtypes.bfloat16

    inv = 1.0 / (10000.0 ** (np.arange(0, RD, 2, dtype=np.float32) / RD))
    ang = np.arange(S, dtype=np.float32)[:, None] * inv[None, :]     # [S, 32]
    cosT = np.cos(ang).T.astype(np.float32)                          # [32, S]
    sinT = np.sin(ang).T.astype(np.float32)
    cos4 = np.ascontiguousarray(np.tile(cosT, (4, 1))).astype(bf16)  # [128,S]
    sin4w = np.ascontiguousarray(np.tile(
        np.concatenate([-sinT, sinT], axis=0), (2, 1))).astype(bf16)  # [128,S]

    in_maps = []
    for c in range(NC):
        b, hg = divmod(c, 4)
        xTb = np.ascontiguousarray(x[b].T.astype(bf16))
        w_big = np.ascontiguousarray(np.concatenate(
            [W_comp,
             W_rope_q[:, hg * 256:(hg + 1) * 256],
             W_rope_k[:, hg * 256:(hg + 1) * 256]], axis=1).astype(bf16))
        w_qk = np.ascontiguousarray(np.concatenate(
            [W_q_dec[:, hg * 256:(hg + 1) * 256],
             W_k_dec[:, hg * 256:(hg + 1) * 256]], axis=1).astype(bf16))
        w_v = np.ascontiguousarray(np.concatenate(
            [W_v_dec[:, hg * 256:(hg + 1) * 256],
             W_v_dec[:, 1024 + hg * 256:1024 + (hg + 1) * 256]],
            axis=1).astype(bf16))
        w_o = np.ascontiguousarray(np.concatenate(
            [W_out[hg * 256:(hg + 1) * 256, :],
             W_out[1024 + hg * 256:1024 + (hg + 1) * 256, :]],
            axis=0).astype(bf16))
        w4 = np.ascontiguousarray(np.concatenate(
            [w_big[k * 128:(k + 1) * 128, 512:640] for k in range(16)],
            axis=1))
        w5 = np.ascontiguousarray(np.concatenate(
            [w_big[k * 128:(k + 1) * 128, 640:768] for k in range(16)],
            axis=1))
        in_maps.append({
            "xT": xTb, "w_big": w_big, "w_qk": w_qk, "w_v": w_v, "w_o": w_o,
            "w4": w4, "w5": w5, "cos4": cos4, "sin4w": sin4w,
        })
    return in_maps


def kernel(x, W_comp, W_q_dec, W_k_dec, W_v_dec, W_rope_q, W_rope_k, W_out,
           _trace=False):
    from concourse import bass_utils

    x = np.asarray(x, np.float32)
    args = [np.asarray(a, np.float32)
            for a in (W_comp, W_q_dec, W_k_dec, W_v_dec,
                      W_rope_q, W_rope_k, W_out)]
    in_maps = _host_shards(x, *args)
    nc = _get_program()
    res = bass_utils.run_bass_kernel_spmd(
        nc, in_maps, core_ids=list(range(NC)), trace=_trace)
    out = np.zeros((B, S, D), np.float32)
    for c in range(NC):
        b = c // 4
        out[b] += res.results[c]["out"].astype(np.float32).T
    if _trace:
        kernel.last_exec_ns = res.exec_time_ns
    return out
